# revision 1
# baseline (speedup 1.0000x reference)
"""VQ codebook nearest-neighbor kernel for TRN2 (8 NeuronCores, data-parallel).

argmin_k ||z - c_k||^2 == argmax_k (2 z.c_k - |c_k|^2), computed per core over
8192 tokens (z sharded along the flattened token dim across the 8 cores; the
1024x64 codebook replicated on every core).

Per 128-token tile:
  - z is split exactly into fp16 hi+lo parts (zh on ACT, zl=z-zh on GPSIMD);
    one PE transpose of the [zh|zl] block yields [zh^T; zl^T] stacked along
    the contract dim.
  - Scores (full f32 accuracy at 2 PE cycles/row instead of f32's 4):
    psum = [zh^T; zl^T] @ [ch^T; ch^T] (128-contract) + zh^T @ cl^T
    (64-contract), where ch/cl are the fp16 hi/lo parts of (2C)^T.
  - A custom single-pass DVE op (running-max Scan + IS_GE select of Idx +
    max-accumulate) returns argmax_f(psum[p,f] - csq[p,f]) straight from
    PSUM -- one 1x pass, no InstMax/InstMaxIndex pair, no SBUF copy.
  - GPSIMD indirect DMA gathers codebook[argmax] rows (one row-offset per
    partition per instruction); batched DMAs move z in / results out.

Cost-model (TimelineSim) estimate: ~99 us per core; steady state is bound by
the DVE argmax pass (~1.2 us per 128x1024 tile).
"""

import sys

sys.path.insert(0, "/opt/trn_rl_repo")

import numpy as np

import concourse.bass as bass
import concourse.bacc as bacc
import concourse.mybir as mybir
from concourse.tile import TileContext
from concourse.bass_utils import run_bass_kernel_spmd
from concourse.masks import make_identity

import concourse.dve_ops as dve_ops
from concourse.dve_ops import DveOp
from concourse.dve_spec import (
    Spec,
    Src0,
    Src1,
    MaxNeg,
    Idx,
    Bin,
    AluOp,
    select,
    maxx,
    lower,
    Scan,
    _has_src1,
)
from concourse.dve_uop import DveOpSpec

# ---------------------------------------------------------------------------
# problem constants (hardcoded per harness contract)
B, T, D = 32, 2048, 64
K = 1024
N_CORES = 8
NTOK = B * T
TOK_PER_CORE = NTOK // N_CORES  # 8192
TILES = TOK_PER_CORE // 128  # 64
GROUP = 2  # tiles per DMA batch
REPEAT = 1  # in-program repetitions of the main loop (for timing experiments)

F32 = mybir.dt.float32

FLT_MAX_NEG = np.float32(-3.4028235e38)


# ---------------------------------------------------------------------------
# custom fused-argmax DVE op: accum_out[p] = argmax_f (in0[p,f] - in1[p,f])
def _argmax_sub_ref(in0, in1, s0, s1, imm2):
    x = (in0.astype(np.float32) - in1.astype(np.float32)).reshape(in0.shape[0], -1)
    run = np.maximum.accumulate(x, axis=1)
    idx = np.arange(x.shape[1], dtype=np.float32)[None, :]
    body = np.where(x >= run, idx, FLT_MAX_NEG).astype(np.float32)
    acc = body.max(axis=1, keepdims=True)
    return body.reshape(in0.shape), acc


def _make_argmax_op():
    for op in dve_ops.OPS:
        if op.name == "ARGMAX_SUB_ANT":
            return op
    x = Bin(AluOp.SUBTRACT, Src0, Src1)
    run = Scan(AluOp.MAX, x)
    body = select(Bin(AluOp.IS_GE, x, run), Idx, MaxNeg)
    spec = Spec(body=body, accum=maxx, accum_init=MaxNeg, reference=_argmax_sub_ref)
    opcode = dve_ops._CUSTOM_DVE_ROW_BASE + len(dve_ops.OPS)
    shas = {}
    for ver in ("v3", "v4"):
        uops = lower(spec, ver=ver)
        s = DveOpSpec(name="ARGMAX_SUB_ANT", opcode=opcode, uops=uops,
                      rd1_en=_has_src1(spec))
        shas[ver] = s.sha(ver)
    op = DveOp("ARGMAX_SUB_ANT", spec, subdim=False, uops_sha=shas)
    dve_ops.OPS.append(op)
    dve_ops.CUSTOM_DVE_SPECS[op.name] = op.spec
    dve_ops._SUB_OPCODE_FOR_NAME[op.name] = opcode
    return op


ARGMAX_SUB = _make_argmax_op()


# ---------------------------------------------------------------------------
def _build_kernel():
    nc = bacc.Bacc(trn_type="TRN2", target_bir_lowering=False, debug=False)
    z = nc.dram_tensor("z", [TOK_PER_CORE, D], F32, kind="ExternalInput")
    cb = nc.dram_tensor("codebook", [K, D], F32, kind="ExternalInput")
    out = nc.dram_tensor("out", [TOK_PER_CORE, D], F32, kind="ExternalOutput")

    with TileContext(nc) as tc:
        with (
            tc.tile_pool(name="const", bufs=1) as cpool,
            tc.tile_pool(name="work", bufs=4) as pool,
            tc.tile_pool(name="scratch", bufs=3) as spool,
            tc.tile_pool(name="psum_s", bufs=3, space="PSUM") as psum_s,
            tc.tile_pool(name="psum_t", bufs=2, space="PSUM") as psum_t,
        ):
            ident = cpool.tile([128, 128], F32)
            make_identity(nc, ident[:])
            FP16 = mybir.dt.float16
            ident16 = cpool.tile([128, 128], FP16, tag="ident16")
            make_identity(nc, ident16[:])

            # ---- preprocessing: C2T = (2*codebook)^T [64, 1024], csq_rep ----
            c2T = cpool.tile([64, K], F32, tag="c2T")
            cbt_all = cpool.tile([128, 8 * D], F32, tag="cb_load")
            nc.sync.dma_start(
                cbt_all[:].rearrange("p (kc d) -> p kc d", kc=8),
                cb[:, :].rearrange("(kc p) d -> p kc d", p=128),
            )
            cb2_all = cpool.tile([128, 8 * D], F32, tag="cb2")
            nc.scalar.mul(cb2_all[:], cbt_all[:], 2.0)
            for kc in range(8):
                pT = psum_t.tile([D, 128], F32, tag="zT")
                nc.tensor.transpose(
                    pT[:], cb2_all[:, kc * D:(kc + 1) * D], ident[:]
                )
                nc.scalar.copy(c2T[:, kc * 128:(kc + 1) * 128], pT[:])

            # csq_rep[p, k] = |c_k|^2 for all p: 0.25*ones^T @ (c2T*c2T)
            c2T_sq = cpool.tile([64, K], F32, tag="c2T_sq")
            nc.vector.tensor_mul(c2T_sq[:], c2T[:], c2T[:])
            qones = cpool.tile([64, 128], F32, tag="qones")
            nc.vector.memset(qones[:], 0.25)
            csq_rep = cpool.tile([128, K], F32, tag="csq_rep")
            for h in range(2):
                pb = psum_s.tile([128, 512], F32, tag="scores")
                nc.tensor.matmul(
                    pb[:], qones[:], c2T_sq[:, h * 512:(h + 1) * 512],
                    start=True, stop=True,
                )
                nc.scalar.copy(csq_rep[:, h * 512:(h + 1) * 512], pb[:])

            # fp16 hi/lo split of (2C)^T; chT2 = [chT; chT] stacked on partitions
            chT = cpool.tile([64, K], FP16, tag="chT")
            clT = cpool.tile([64, K], FP16, tag="clT")
            nc.scalar.copy(chT[:], c2T[:])
            nc.vector.tensor_sub(clT[:], c2T[:], chT[:])
            chT2 = cpool.tile([128, K], FP16, tag="chT2")
            nc.sync.dma_start(chT2[0:64, :], chT[:])
            nc.sync.dma_start(chT2[64:128, :], chT[:])

            # ---- main loop over groups of GROUP tiles ----
            n_groups = TILES // GROUP
            for g_rep in range(REPEAT * n_groups):
                g = g_rep % n_groups
                tok0 = g * GROUP * 128
                # batched z load: [128, GROUP, 64]; token (g*GROUP+k)*128+p -> [p, k, :]
                zsb = pool.tile([128, GROUP * D], F32, tag="zsb")
                nc.sync.dma_start(
                    zsb[:].rearrange("p (k d) -> p k d", k=GROUP),
                    z[tok0:tok0 + GROUP * 128, :].rearrange(
                        "(k p) d -> p k d", p=128
                    ),
                )
                idxf = pool.tile([128, GROUP], F32, tag="idxf")
                gout = pool.tile([128, GROUP * D], F32, tag="gout")
                # group-batched fp16 hi/lo split of z
                zh_g = pool.tile([128, GROUP * D], FP16, tag="zh_g")
                zl_g = pool.tile([128, GROUP * D], FP16, tag="zl_g")
                nc.scalar.copy(zh_g[:], zsb[:])
                nc.gpsimd.tensor_sub(zl_g[:], zsb[:], zh_g[:])
                for k in range(GROUP):
                    ks = slice(k * D, (k + 1) * D)
                    pzT = psum_t.tile([128, 128], FP16, tag="zT")
                    nc.tensor.transpose(pzT[0:64, :], zh_g[:, ks], ident16[:])
                    nc.tensor.transpose(pzT[64:128, :], zl_g[:, ks], ident16[:])
                    zaT = pool.tile([128, 128], FP16, tag="zaT")
                    nc.scalar.copy(zaT[:], pzT[:])

                    ps = psum_s.tile([128, K], F32, tag="scores")
                    for h in range(2):
                        hs = slice(h * 512, (h + 1) * 512)
                        nc.tensor.matmul(ps[:, hs], zaT[:, :], chT2[:, hs],
                                         start=True, stop=False)
                        nc.tensor.matmul(ps[:, hs], zaT[0:64, :], clT[:, hs],
                                         start=False, stop=True)
                    scratch = spool.tile([128, K], F32, tag="amx_scratch")
                    nc.vector._custom_dve(
                        ARGMAX_SUB,
                        out=scratch[:],
                        in0=ps[:],
                        in1=csq_rep[:],
                        accum_out=idxf[:, k:k + 1],
                    )
                # convert to int32 and gather per tile (decoupled per k)
                idxi = pool.tile([128, GROUP], mybir.dt.int32, tag="idxi")
                for k in range(GROUP):
                    nc.scalar.copy(idxi[:, k:k + 1], idxf[:, k:k + 1])
                    nc.gpsimd.indirect_dma_start(
                        out=gout[:].rearrange("p (k d) -> p k d", k=GROUP)[:, k, :],
                        out_offset=None,
                        in_=cb[:, :],
                        in_offset=bass.IndirectOffsetOnAxis(
                            ap=idxi[:, k:k + 1], axis=0
                        ),
                    )
                # per-tile store
                for k in range(GROUP):
                    nc.sync.dma_start(
                        out[tok0 + k * 128:tok0 + (k + 1) * 128, :],
                        gout[:].rearrange("p (k d) -> p k d", k=GROUP)[:, k, :],
                    )

    nc.compile()
    return nc


_NC_CACHE = None


def _get_nc():
    global _NC_CACHE
    if _NC_CACHE is None:
        _NC_CACHE = _build_kernel()
    return _NC_CACHE


def kernel(z: np.ndarray, codebook: np.ndarray) -> np.ndarray:
    nc = _get_nc()
    z = np.ascontiguousarray(z, dtype=np.float32)
    codebook = np.ascontiguousarray(codebook, dtype=np.float32)
    z_flat = z.reshape(-1, D)
    shards = np.split(z_flat, N_CORES, axis=0)
    in_maps = [{"z": s, "codebook": codebook} for s in shards]
    res = run_bass_kernel_spmd(nc, in_maps, core_ids=list(range(N_CORES)))
    out = np.concatenate([res.results[c]["out"] for c in range(N_CORES)], axis=0)
    return out.reshape(z.shape)

